# revision 1
# baseline (speedup 1.0000x reference)
"""Cost-volume concat kernel for Trainium2 (8 NeuronCores, SPMD).

Problem: left/right (B=4, C=32, H=64, W=128) f32 ->
         out (B, 2C, D=48, H, W) where
  out[b, c,    d, h, w] = left [b, c, h, w]     * (w >= d)
  out[b, C+c,  d, h, w] = right[b, c, h, w - d] * (w >= d)

Sharding: 8 cores = 4 batches x 2 disparity-halves (d0 in {0, 24}).
All cores run an IDENTICAL program (single SPMD NEFF); the d0 shift is
absorbed host-side by pre-shifting the left input by d0 columns and
stitching the per-core output back with a d0 column offset:

  core (b, q), d0 = 24q, level i in [0, 24):
    xl[c,h,w]      = left[b,c,h,w+d0]  (zero-padded tail)
    xr[c,h,24+w]   = right[b,c,h,w]    (24 leading zero columns baked in)
    yl[c, i, h, w] = xl[c,h,w] * (w >= i)
    yr[c, i, h, w] = xr[c,h,w-i] * (w >= i)
  host: out[b, 0:C, d0+i, h, d0+w] = yl[c, i, h, w]
        out[b, C:,  d0+i, h, d0+w] = yr[c, i, h, w]   (rest stays zero)

The kernel is pure DMA (no compute):
  - right half: full-width sliding-window reads from the padded tile
    (the pad supplies the w < i zeros), 24 x 1MB stores;
  - left half: the w >= i tail only -- output buffers are zero-filled
    by the runtime (run_bass_kernel_spmd pre-zeros ExternalOutputs on
    both the native and the PJRT/axon path), so masked zeros need no
    write at all;
  - every DMA carries at most one sync wait (walrus's HWDGE direct2d
    limit): data deps exist only against the two input loads, which the
    first DMA of each ring observes once.
"""

import sys

for _p in ("/opt/trn_rl_repo",):
    if _p not in sys.path:
        sys.path.append(_p)

import numpy as np

import concourse.bass as bass
import concourse.mybir as mybir
import concourse.tile as tile
from concourse.bass_utils import run_bass_kernel_spmd

B, C, H, W = 4, 32, 64, 128
D = 48
NCORES = 8
DL = D // 2          # 24 disparity levels per core
PAD = DL             # zero-pad columns for the shifted right-half reads
ROWS = C * H // 128  # 16 (c,h)-rows per SBUF partition

_F32 = mybir.dt.float32

_NC_CACHE = {}


class _SplitDrainTC(tile.TileContext):
    """TileContext whose kernel-tail drain legalizes to <=1 sem wait per
    instruction: this walrus pipeline (policy 0, no sync passes) rejects
    any instruction carrying more than one sync wait, and the stock
    _drain_and_barrier puts every outstanding DMA-lane sem on one Drain.
    We keep the first wait on the drain and chain the rest through extra
    single-wait drains on the same (in-order) SP queue."""

    def _drain_and_barrier(self, tick_clock, wait_clock):
        from concourse.vector_clock import ScopedClock

        nc = self.nc
        drain_inst = nc.sync.drain(fusable=False)
        wait_clock.add_sem_waits(
            drain_inst.ins, ScopedClock({None: tick_clock.global_clock})
        )
        si = drain_inst.ins.sync_info
        if si is not None and len(si.on_wait) > 1:
            waits = list(si.on_wait)
            drain_inst.ins.sync_info = mybir.SyncInfo(
                on_wait=[waits[0]], on_update=list(si.on_update)
            )
            for w in waits[1:]:
                extra = nc.sync.drain(fusable=False)
                extra.ins.sync_info = mybir.SyncInfo(on_wait=[w], on_update=[])

        nc.all_engine_barrier()
        assert self.sems is not None
        popped = nc._tile_sem_poison_stack.pop()
        assert popped is self._sem_poison
        nc.clear_and_free_semaphores(list(self.sems.allocated().values()))
        nc.all_engine_barrier()


def _build_nc():
    """One SPMD program for every core; ~52 instructions, no control flow."""
    nc = bass.Bass()
    xl = nc.dram_tensor("xl", [C, H, W], _F32, kind="ExternalInput")
    xr = nc.dram_tensor("xr", [C, H, PAD + W], _F32, kind="ExternalInput")
    # Two outputs, one per HWDGE ring: a single shared output tensor makes
    # Tile emit cross-engine WAW waits on every DMA (walrus rejects >1 sync
    # wait per HWDGE DMA); disjoint tensors keep each ring's DMAs dep-free.
    yl = nc.dram_tensor("yl", [C, DL, H, W], _F32, kind="ExternalOutput")
    yr = nc.dram_tensor("yr", [C, DL, H, W], _F32, kind="ExternalOutput")

    with _SplitDrainTC(nc) as tc:
        with tc.tile_pool(name="pool", bufs=1) as pool:
            # Partition p holds 16 consecutive (c,h) rows -> every DMA AP
            # collapses to <=3 dims with contiguous inner runs.
            lt = pool.tile([128, ROWS, W], _F32, name="lt")
            rt = pool.tile([128, ROWS, PAD + W], _F32, name="rt")

            # Loads ride the same two HWDGE rings as the stores: SWDGE lanes
            # would add two more sems to the kernel-tail drain, which only
            # supports 8 sync waits.
            nc.sync.dma_start(lt[:], xl[:])
            nc.scalar.dma_start(rt[:], xr[:])

            for i in range(DL):
                # Right half (ACT ring): full 512B rows; the window start
                # walks back through the pad, which supplies the zeros.
                nc.scalar.dma_start(
                    yr[:, i, :, :], rt[:, :, PAD - i:PAD - i + W]
                )
                # Left half (SP ring): only the unmasked w >= i tail; the
                # pre-zeroed output keeps the masked prefix at zero.
                if i == 0:
                    nc.sync.dma_start(yl[:, 0, :, :], lt[:])
                else:
                    nc.sync.dma_start(yl[:, i, :, i:], lt[:, :, i:])
    return nc


def _get_nc():
    if "nc" not in _NC_CACHE:
        _NC_CACHE["nc"] = _build_nc()
    return _NC_CACHE["nc"]


def _run(left, right, **spmd_kwargs):
    left = np.ascontiguousarray(np.asarray(left), dtype=np.float32)
    right = np.ascontiguousarray(np.asarray(right), dtype=np.float32)

    in_maps = []
    for k in range(NCORES):
        b, q = divmod(k, 2)
        d0 = DL * q
        xl = np.zeros((C, H, W), np.float32)
        xl[:, :, :W - d0] = left[b, :, :, d0:]
        xr = np.zeros((C, H, PAD + W), np.float32)
        xr[:, :, PAD:] = right[b]
        in_maps.append({"xl": xl, "xr": xr})

    res = run_bass_kernel_spmd(
        _get_nc(), in_maps, core_ids=list(range(NCORES)), **spmd_kwargs
    )

    out = np.zeros((B, 2 * C, D, H, W), np.float32)
    for k in range(NCORES):
        b, q = divmod(k, 2)
        d0 = DL * q
        out[b, 0:C, d0:d0 + DL, :, d0:] = res.results[k]["yl"][:, :, :, :W - d0]
        out[b, C:, d0:d0 + DL, :, d0:] = res.results[k]["yr"][:, :, :, :W - d0]
    return out, res


def kernel(left, right):
    out, _ = _run(left, right)
    return out



# revision 2
# speedup vs baseline: 1.6807x; 1.6807x over previous
"""Cost-volume concat kernel for Trainium2 (8 NeuronCores, SPMD).

Problem: left/right (B=4, C=32, H=64, W=128) f32 ->
         out (B, 2C, D=48, H, W) where
  out[b, c,    d, h, w] = left [b, c, h, w]     * (w >= d)
  out[b, C+c,  d, h, w] = right[b, c, h, w - d] * (w >= d)

Sharding: 8 cores = 4 batches x 2 H-halves (32 rows each); every core
builds all 48 disparity levels for both halves -> perfectly balanced,
identical SPMD program, no host-side shifting.

Layout: the kernel works in a channel-innermost frame, (h, w, c) for
inputs and (d, h, w, c) for outputs; the host transposes in/out.  This
makes the masked zero region of level d a contiguous d*128B prefix of
each (d, h) row-group, so it can be skipped entirely (the runtime
pre-zeros ExternalOutputs on both the native and the PJRT/axon path)
while the data runs stay (128-d)*128B >= 10KB.  In the natural (d, h, w)
layout the tail runs are (128-d)*4B < 512B, which the DMA moves at half
throughput (sub-512B runs force read-modify-write internally).

The kernel is pure DMA (no compute):
  - level d, left : y[d, h, w>=d, c] <- xl[h, w>=d, c]   (suffix window)
  - level d, right: y[d, h, w>=d, c] <- xr[h, w-d,  c]   (prefix window)
  - two HWDGE rings (SP carries left, ACT carries right), one load plus
    48 stores each; each ring's DMAs touch only its own tile and output
    tensor, so no DMA carries more than one sync wait (walrus's HWDGE
    direct2d limit).
"""

import sys

for _p in ("/opt/trn_rl_repo",):
    if _p not in sys.path:
        sys.path.append(_p)

import numpy as np

import concourse.bass as bass
import concourse.mybir as mybir
import concourse.tile as tile
from concourse.bass_utils import run_bass_kernel_spmd

B, C, H, W = 4, 32, 64, 128
D = 48
NCORES = 8
HH = H // 2          # 32 h-rows per core

_F32 = mybir.dt.float32

_NC_CACHE = {}


class _SplitDrainTC(tile.TileContext):
    """TileContext whose kernel-tail drain legalizes to <=1 sem wait per
    instruction: this walrus pipeline (policy 0, no sync passes) rejects
    any instruction carrying more than one sync wait, and the stock
    _drain_and_barrier puts every outstanding DMA-lane sem on one Drain.
    We keep the first wait on the drain and chain the rest through extra
    single-wait drains on the same (in-order) SP queue."""

    def _drain_and_barrier(self, tick_clock, wait_clock):
        from concourse.vector_clock import ScopedClock

        nc = self.nc
        drain_inst = nc.sync.drain(fusable=False)
        wait_clock.add_sem_waits(
            drain_inst.ins, ScopedClock({None: tick_clock.global_clock})
        )
        si = drain_inst.ins.sync_info
        if si is not None and len(si.on_wait) > 1:
            waits = list(si.on_wait)
            drain_inst.ins.sync_info = mybir.SyncInfo(
                on_wait=[waits[0]], on_update=list(si.on_update)
            )
            for w in waits[1:]:
                extra = nc.sync.drain(fusable=False)
                extra.ins.sync_info = mybir.SyncInfo(on_wait=[w], on_update=[])

        nc.all_engine_barrier()
        assert self.sems is not None
        popped = nc._tile_sem_poison_stack.pop()
        assert popped is self._sem_poison
        nc.clear_and_free_semaphores(list(self.sems.allocated().values()))
        nc.all_engine_barrier()


def _build_nc():
    """One SPMD program for every core; 98 DMAs, no control flow."""
    nc = bass.Bass()
    xl = nc.dram_tensor("xl", [HH, W, C], _F32, kind="ExternalInput")
    xr = nc.dram_tensor("xr", [HH, W, C], _F32, kind="ExternalInput")
    # Two outputs, one per HWDGE ring: a single shared output tensor makes
    # Tile emit cross-engine WAW waits on every DMA (walrus rejects >1 sync
    # wait per HWDGE DMA); disjoint tensors keep each ring's DMAs dep-free.
    yl = nc.dram_tensor("yl", [D, HH, W, C], _F32, kind="ExternalOutput")
    yr = nc.dram_tensor("yr", [D, HH, W, C], _F32, kind="ExternalOutput")

    with _SplitDrainTC(nc) as tc:
        with tc.tile_pool(name="pool", bufs=1) as pool:
            # Partition p holds h-row p: the (w, c) plane is one 16KB run.
            lt = pool.tile([HH, W, C], _F32, name="lt")
            rt = pool.tile([HH, W, C], _F32, name="rt")

            nc.sync.dma_start(lt[:], xl[:])
            nc.scalar.dma_start(rt[:], xr[:])

            for d in range(D):
                # Left (SP ring): suffix window, in place.
                nc.sync.dma_start(yl[d, :, d:, :], lt[:, d:, :])
                # Right (ACT ring): prefix window shifted to column d.
                nc.scalar.dma_start(yr[d, :, d:, :], rt[:, : W - d, :])
    return nc


def _get_nc():
    if "nc" not in _NC_CACHE:
        _NC_CACHE["nc"] = _build_nc()
    return _NC_CACHE["nc"]


def _run(left, right, **spmd_kwargs):
    left = np.ascontiguousarray(np.asarray(left), dtype=np.float32)
    right = np.ascontiguousarray(np.asarray(right), dtype=np.float32)

    in_maps = []
    for k in range(NCORES):
        b, s = divmod(k, 2)
        rows = slice(HH * s, HH * (s + 1))
        xl = np.ascontiguousarray(left[b, :, rows, :].transpose(1, 2, 0))
        xr = np.ascontiguousarray(right[b, :, rows, :].transpose(1, 2, 0))
        in_maps.append({"xl": xl, "xr": xr})

    res = run_bass_kernel_spmd(
        _get_nc(), in_maps, core_ids=list(range(NCORES)), **spmd_kwargs
    )

    out = np.empty((B, 2 * C, D, H, W), np.float32)
    for k in range(NCORES):
        b, s = divmod(k, 2)
        rows = slice(HH * s, HH * (s + 1))
        # (d, h, w, c) -> (c, d, h, w)
        out[b, 0:C, :, rows, :] = res.results[k]["yl"].transpose(3, 0, 1, 2)
        out[b, C:, :, rows, :] = res.results[k]["yr"].transpose(3, 0, 1, 2)
    return out, res


def kernel(left, right):
    out, _ = _run(left, right)
    return out


# revision 4
# speedup vs baseline: 1.7346x; 1.0321x over previous
"""Cost-volume concat kernel for Trainium2 (8 NeuronCores, SPMD).

Problem: left/right (B=4, C=32, H=64, W=128) f32 ->
         out (B, 2C, D=48, H, W) where
  out[b, c,    d, h, w] = left [b, c, h, w]     * (w >= d)
  out[b, C+c,  d, h, w] = right[b, c, h, w - d] * (w >= d)

Sharding: 8 cores = 4 batches x 2 H-halves (32 rows each); every core
builds all 48 disparity levels for both halves -> perfectly balanced,
identical SPMD program, no host-side shifting.

Layout: the kernel works in a channel-innermost frame, (h, w, c) for
inputs and (d, h, w, c) for outputs; the host transposes in/out.  This
makes the masked zero region of level d a contiguous d*128B prefix of
each (d, h) row-group, so it can be skipped entirely (the runtime
pre-zeros ExternalOutputs on both the native and the PJRT/axon path)
while the data runs stay (128-d)*128B >= 10KB.  In the natural (d, h, w)
layout the tail runs are (128-d)*4B < 512B, which the DMA moves at half
throughput (sub-512B runs force read-modify-write internally).

The kernel is pure DMA (no compute, no SBUF staging): every store reads
its window straight from the input DRAM tensor,
  - level d, left : y[d, h, w>=d, c] <- xl[h, w>=d, c]   (suffix window)
  - level d, right: y[d, h, w>=d, c] <- xr[h, w-d,  c]   (prefix window)
on two HWDGE rings (SP carries left, ACT carries right), 48 stores each.
Inputs have no producer and the output tensors are ring-disjoint, so no
DMA carries any sync wait (walrus's HWDGE direct2d limit is <=1).
"""

import sys

for _p in ("/opt/trn_rl_repo",):
    if _p not in sys.path:
        sys.path.append(_p)

import numpy as np

import concourse.bass as bass
import concourse.mybir as mybir
import concourse.tile as tile
from concourse.bass_utils import run_bass_kernel_spmd

B, C, H, W = 4, 32, 64, 128
D = 48
NCORES = 8
HH = H // 2          # 32 h-rows per core

_F32 = mybir.dt.float32

_NC_CACHE = {}


class _SplitDrainTC(tile.TileContext):
    """TileContext whose kernel-tail drain legalizes to <=1 sem wait per
    instruction: this walrus pipeline (policy 0, no sync passes) rejects
    any instruction carrying more than one sync wait, and the stock
    _drain_and_barrier puts every outstanding DMA-lane sem on one Drain.
    We keep the first wait on the drain and chain the rest through extra
    single-wait drains on the same (in-order) SP queue."""

    def _drain_and_barrier(self, tick_clock, wait_clock):
        from concourse.vector_clock import ScopedClock

        nc = self.nc
        drain_inst = nc.sync.drain(fusable=False)
        wait_clock.add_sem_waits(
            drain_inst.ins, ScopedClock({None: tick_clock.global_clock})
        )
        si = drain_inst.ins.sync_info
        if si is not None and len(si.on_wait) > 1:
            waits = list(si.on_wait)
            drain_inst.ins.sync_info = mybir.SyncInfo(
                on_wait=[waits[0]], on_update=list(si.on_update)
            )
            for w in waits[1:]:
                extra = nc.sync.drain(fusable=False)
                extra.ins.sync_info = mybir.SyncInfo(on_wait=[w], on_update=[])

        nc.all_engine_barrier()
        assert self.sems is not None
        popped = nc._tile_sem_poison_stack.pop()
        assert popped is self._sem_poison
        nc.clear_and_free_semaphores(list(self.sems.allocated().values()))
        nc.all_engine_barrier()


def _build_nc():
    """One SPMD program for every core; 98 DMAs, no control flow."""
    nc = bass.Bass()
    xl = nc.dram_tensor("xl", [HH, W, C], _F32, kind="ExternalInput")
    xr = nc.dram_tensor("xr", [HH, W, C], _F32, kind="ExternalInput")
    # Two outputs, one per HWDGE ring: a single shared output tensor makes
    # Tile emit cross-engine WAW waits on every DMA (walrus rejects >1 sync
    # wait per HWDGE DMA); disjoint tensors keep each ring's DMAs dep-free.
    yl = nc.dram_tensor("yl", [D, HH, W, C], _F32, kind="ExternalOutput")
    yr = nc.dram_tensor("yr", [D, HH, W, C], _F32, kind="ExternalOutput")

    with _SplitDrainTC(nc):
        for d in range(D):
            # Left (SP ring): suffix window, in place.
            nc.sync.dma_start(yl[d, :, d:, :], xl[:, d:, :])
            # Right (ACT ring): prefix window shifted to column d.
            nc.scalar.dma_start(yr[d, :, d:, :], xr[:, : W - d, :])
    return nc


def _get_nc():
    if "nc" not in _NC_CACHE:
        _NC_CACHE["nc"] = _build_nc()
    return _NC_CACHE["nc"]


def _run(left, right, **spmd_kwargs):
    left = np.ascontiguousarray(np.asarray(left), dtype=np.float32)
    right = np.ascontiguousarray(np.asarray(right), dtype=np.float32)

    in_maps = []
    for k in range(NCORES):
        b, s = divmod(k, 2)
        rows = slice(HH * s, HH * (s + 1))
        xl = np.ascontiguousarray(left[b, :, rows, :].transpose(1, 2, 0))
        xr = np.ascontiguousarray(right[b, :, rows, :].transpose(1, 2, 0))
        in_maps.append({"xl": xl, "xr": xr})

    res = run_bass_kernel_spmd(
        _get_nc(), in_maps, core_ids=list(range(NCORES)), **spmd_kwargs
    )

    out = np.empty((B, 2 * C, D, H, W), np.float32)
    for k in range(NCORES):
        b, s = divmod(k, 2)
        rows = slice(HH * s, HH * (s + 1))
        # (d, h, w, c) -> (c, d, h, w)
        out[b, 0:C, :, rows, :] = res.results[k]["yl"].transpose(3, 0, 1, 2)
        out[b, C:, :, rows, :] = res.results[k]["yr"].transpose(3, 0, 1, 2)
    return out, res


def kernel(left, right):
    out, _ = _run(left, right)
    return out


# revision 12
# speedup vs baseline: 7.8333x; 4.5159x over previous
"""Cost-volume concat kernel for Trainium2 (8 NeuronCores, SPMD).

Problem: left/right (B=4, C=32, H=64, W=128) f32 ->
         out (B, 2C, D=48, H, W) where
  out[b, c,    d, h, w] = left [b, c, h, w]     * (w >= d)
  out[b, C+c,  d, h, w] = right[b, c, h, w - d] * (w >= d)

Sharding: 8 cores = 4 batches x 2 level-halves (d in [24q, 24q+24)); every
core builds both the left and right channel halves for its 24 levels.

The stores go through gpsimd kv_writeback instead of plain DMA.  Work in a
channel-innermost frame: per core and half, y[dd, g, w, cg] with the 2048
(c, h) pairs split into g in [0,16) groups of cg in [0,128).  kv_writeback
writes, per batch entry b and partition p, dho runs of ncn contiguous
elements at per-batch column offset ctx_idx[b]:

  dst[b, p, j, idx[b] : idx[b]+ncn] , src[p, (j*batch_step + b)*ncn + k]

Mapping (ncn = 128, dho = 16, batch = 24 levels, partition p = g*8 + s):
  LEFT : chunk n = s*16 + j + dd at idx[dd] = dd*128; src block t = j + dd;
         partition (g,s) holds blocks P[t] = content[g, w = s*16 + t]
         (zero past w=127).  Writes n < dd are skipped => the masked
         prefix stays at the runtime's pre-zeroed fill; writes n >= 128
         overflow into the next row-group's zero prefix with zero data.
  RIGHT: stored w-REVERSED (host un-reverses): chunk n = s*16 + j at
         idx = 0; P[t] = content[g, w = 127 - s*16 - t] (zero for
         negative index), which makes the per-level source shift t = j+dd
         land on level-independent partition content, and the reversed
         mask zeros come from the same padding.

Level-base 24q is baked host-side: the left content is pre-shifted by 24q
columns, the right content is unshifted; all cores run an identical SPMD
program.  The left half's per-level dst shift idx[dd] = dd*128 is linear
in the batch index, so it is absorbed into batch_stride_bytes (LVL+128
elems per level) and the ctx_idxs are all-zero for both halves -- the idx
tile is a DVE memset, not a load.

Why kv_writeback: each instruction moves batch*2048 512B-runs but the DGE
costs descriptors per 16-partition stripe, so the store phase costs ~16x
less DMA-engine time than HWDGE dma_start (~9us instead of ~114us for the
two halves) and the kernel becomes load-bound (~5MB of SBUF tiles at
~360B/ns).  Each half is ONE batch=24 writeback whose ~2us desc-gen hides
in the gap between its tile landing and the DMA engines draining the
loads; the 3073-descriptor ring entry needs the enlarged
dynamic_dma_scratch_size.
"""

import sys

for _p in ("/opt/trn_rl_repo",):
    if _p not in sys.path:
        sys.path.append(_p)

import numpy as np

import concourse.bacc as bacc
import concourse.bass as bass
import concourse.mybir as mybir

B, C, H, W = 4, 32, 64, 128
D = 48
NCORES = 8
G, S, DHO, NCN = 16, 8, 16, 128
LCORE = D // 2                 # 24 levels per core
SPAN = DHO + LCORE - 1         # 39 blocks of 128 elems per partition
FREE = SPAN * NCN              # 4992 f32 per partition
LVL = G * 128 * NCN            # 262144 elems per level
NPAD = 8192                    # dram overflow pad (elems)
NCTX = 4096

_F32 = mybir.dt.float32
_I32 = mybir.dt.int32

_NC_CACHE = {}


INSTS = [(0, 7), (7, 7), (14, 7), (21, 3)]   # (level base, batch) per inst


def _build_nc():
    """One SPMD program for every core: 2 HWDGE loads, a memset'd zero
    idx tile, 8 kv_writebacks (4 per half; each inst's ~900 descriptors
    must fit the default 1024-entry SWDGE ring -- the ucode carveout size
    is fixed, enlarging dynamic_dma_scratch_size crashes on HW)."""
    nc = bacc.Bacc()
    tl_d = nc.dram_tensor("tl_d", [128, FREE], _F32, kind="ExternalInput")
    tr_d = nc.dram_tensor("tr_d", [128, FREE], _F32, kind="ExternalInput")
    yl = nc.dram_tensor("yl", [LCORE * LVL + NPAD], _F32, kind="ExternalOutput")
    yr = nc.dram_tensor("yr", [LCORE * LVL + NPAD], _F32, kind="ExternalOutput")

    with (
        nc.Block() as block,
        nc.sbuf_tensor("tl", [128, FREE], _F32) as tl,
        nc.sbuf_tensor("tr", [128, FREE], _F32) as tr,
        nc.sbuf_tensor("ix", [128, 8], _I32) as ix,
        nc.semaphore("ld_l") as ld_l,
        nc.semaphore("ld_r") as ld_r,
        nc.semaphore("ixs") as ixs,
        nc.semaphore("wbd") as wbd,
    ):

        @block.sync
        def _(sync):
            sync.dma_start(tl[:, :], tl_d[:, :]).then_inc(ld_l, 16)

        @block.scalar
        def _(scalar):
            scalar.dma_start(tr[:, :], tr_d[:, :]).then_inc(ld_r, 16)

        @block.vector
        def _(vector):
            vector.memset(ix[:, :], 0).then_inc(ixs, 1)

        @block.gpsimd
        def _(gp):
            def wbs(y, t, lvl_stride):
                for l0, L in INSTS:
                    out_ap = bass.AP(
                        y,
                        l0 * lvl_stride,
                        [[lvl_stride, L], [DHO * NCN, 128], [NCN, DHO], [1, NCTX]],
                    )
                    in_ap = bass.AP(
                        t, l0 * NCN, [[FREE, 128], [NCN, DHO], [NCN, L], [1, NCN]]
                    )
                    gp.kv_writeback(out_ap, in_ap, ix[:, :L]).then_inc(wbd, 16)

            gp.wait_ge(ixs, 1)
            gp.wait_ge(ld_l, 16)
            wbs(yl, tl, LVL + NCN)   # left: idx absorbed into batch stride
            gp.wait_ge(ld_r, 16)
            wbs(yr, tr, LVL)         # right: plain level stride
            gp.wait_ge(wbd, 16 * 8)

    nc.compile()
    return nc


def _get_nc():
    if "nc" not in _NC_CACHE:
        _NC_CACHE["nc"] = _build_nc()
    return _NC_CACHE["nc"]


# Partition content gather indices, precomputed once:
#   left : block w-index  M[s, t] = s*16 + t           (invalid -> zero)
#   right: block w-index  M[s, t] = 127 - s*16 - t     (invalid -> zero)
_T = np.arange(SPAN)
_ML = (np.arange(S) * DHO)[:, None] + _T[None, :]            # [S, SPAN]
_MR = 127 - _ML
_VL, _VR = _ML < 128, _MR >= 0


def _content_tiles(half_gw, ml, valid):
    """half_gw: [G, 128(w), 128(cg)] -> SBUF tile [128, FREE]."""
    t = half_gw[:, np.clip(ml, 0, 127), :]                   # [G, S, SPAN, 128]
    t *= valid[None, :, :, None]
    return np.ascontiguousarray(t.reshape(128, FREE))


def _run(left, right, **spmd_kwargs):
    from concourse.bass_utils import run_bass_kernel_spmd

    left = np.ascontiguousarray(np.asarray(left), dtype=np.float32)
    right = np.ascontiguousarray(np.asarray(right), dtype=np.float32)

    in_maps = []
    for k in range(NCORES):
        b, q = divmod(k, 2)
        # [g, w, cg] frames; ch = c*64 + h = g*128 + cg
        lw = left[b].reshape(G, 128, W).transpose(0, 2, 1)
        rw = right[b].reshape(G, 128, W).transpose(0, 2, 1)
        cl = np.zeros((G, 128, 128), np.float32)
        cl[:, : W - 24 * q, :] = lw[:, 24 * q :, :]          # bake level base
        in_maps.append(
            {
                "tl_d": _content_tiles(cl, _ML, _VL),
                "tr_d": _content_tiles(rw, _MR, _VR),
            }
        )

    res = run_bass_kernel_spmd(
        _get_nc(), in_maps, core_ids=list(range(NCORES)), **spmd_kwargs
    )

    out = np.zeros((B, 2 * C, D, H, W), np.float32)
    for k in range(NCORES):
        b, q = divmod(k, 2)
        wlim = W - 24 * q
        dsl = slice(24 * q, 24 * q + LCORE)
        wsl = slice(24 * q, W)

        def stitch(flat, rev):
            y = flat[: LCORE * LVL].reshape(LCORE, G, 128, 128)
            y = y[:, :, 128 - wlim :, :][:, :, ::-1, :] if rev else y[:, :, :wlim, :]
            # [dd, g, wl, cg] -> [c, dd, h, wl]
            y = y.transpose(1, 3, 0, 2).reshape(C, H, LCORE, wlim)
            return y.transpose(0, 2, 1, 3)

        out[b, 0:C, dsl, :, wsl] = stitch(res.results[k]["yl"], False)
        out[b, C:, dsl, :, wsl] = stitch(res.results[k]["yr"], True)
    return out, res


def kernel(left, right):
    out, _ = _run(left, right)
    return out
